# revision 3
# baseline (speedup 1.0000x reference)
"""Top-8-per-row kernel for x[2048, 32768] fp32 on 8 TRN2 NeuronCores.

Strategy: data-parallel over rows (256 rows/core = 2 partition blocks of
128). Stream column tiles into SBUF, use the DVE InstMax (top-8 per
partition, descending) per tile, then a final InstMax over the per-tile
candidates, then reverse to ascending order and DMA out.
"""

from contextlib import ExitStack

import numpy as np

import concourse.bass as bass
import concourse.tile as tile
from concourse import bacc, mybir
from concourse.bass_utils import run_bass_kernel_spmd

B = 2048
N = 32768
K = 8
N_CORES = 8
ROWS_PER_CORE = B // N_CORES  # 256
P = 128
N_BLOCKS = ROWS_PER_CORE // P  # 2
TILE_C = 16384  # InstMax max free size
N_TILES = N // TILE_C  # 2
F32 = mybir.dt.float32


def _build() -> bass.Bass:
    nc = bacc.Bacc(
        "TRN2", target_bir_lowering=False, debug=False, num_devices=N_CORES
    )
    x = nc.dram_tensor("x", [ROWS_PER_CORE, N], F32, kind="ExternalInput").ap()
    out = nc.dram_tensor("out", [ROWS_PER_CORE, K], F32, kind="ExternalOutput").ap()

    with ExitStack() as ctx:
        tc = ctx.enter_context(tile.TileContext(nc))
        data_pool = ctx.enter_context(tc.tile_pool(name="data", bufs=2))
        small_pool = ctx.enter_context(tc.tile_pool(name="small", bufs=2 * N_BLOCKS))

        for b in range(N_BLOCKS):
            rows = slice(b * P, (b + 1) * P)
            cands = small_pool.tile([P, K * N_TILES], F32, tag="cands")
            for t in range(N_TILES):
                d = data_pool.tile([P, TILE_C], F32, tag="data")
                nc.sync.dma_start(d[:], x[rows, t * TILE_C : (t + 1) * TILE_C])
                nc.vector.max(cands[:, t * K : (t + 1) * K], d[:])
            top = small_pool.tile([P, K], F32, tag="top")
            nc.vector.max(top[:], cands[:])
            asc = small_pool.tile([P, K], F32, tag="asc")
            nc.vector.tensor_copy(asc[:], top[:, ::-1])
            nc.sync.dma_start(out[rows, :], asc[:])

    nc.compile()
    return nc


def kernel(x: np.ndarray, k) -> np.ndarray:
    k = int(np.asarray(k))
    assert k == K, f"kernel hardcoded for k={K}, got {k}"
    assert x.shape == (B, N), x.shape
    x = np.ascontiguousarray(x, dtype=np.float32)

    nc = _build()
    in_maps = [
        {"x": x[c * ROWS_PER_CORE : (c + 1) * ROWS_PER_CORE]} for c in range(N_CORES)
    ]
    res = run_bass_kernel_spmd(nc, in_maps, list(range(N_CORES)))
    out = np.concatenate([res.results[c]["out"] for c in range(N_CORES)], axis=0)
    return np.asarray(out, dtype=np.float32)


if __name__ == "__main__":
    rng = np.random.default_rng(0)
    xs = rng.standard_normal((B, N), dtype=np.float32)
    got = kernel(xs, 8)
    want = np.sort(xs, axis=1)[:, -K:]
    err = np.max(np.abs(got - want))
    print("absmax err:", err)


# revision 21
# speedup vs baseline: 1.1522x; 1.1522x over previous
"""Top-8-per-row kernel for x[2048, 32768] fp32 on 8 TRN2 NeuronCores.

Strategy: data-parallel over rows (256 rows/core = 2 partition blocks of
128). Stream column tiles into SBUF, use the DVE InstMax (top-8 per
partition, descending) per tile, then a final InstMax over the per-tile
candidates, then reverse to ascending order and DMA out.
"""

from contextlib import ExitStack

import numpy as np

import concourse.bass as bass
import concourse.tile as tile
from concourse import bacc, mybir
from concourse.bass_utils import run_bass_kernel_spmd

B = 2048
N = 32768
K = 8
N_CORES = 8
ROWS_PER_CORE = B // N_CORES  # 256
P = 128
N_BLOCKS = ROWS_PER_CORE // P  # 2
# Column tile sizes per 128-row block (max8 granularity = DMA
# granularity). 4096 cols -> 16KB partition lines, which keep all 16
# SDMA engines at line rate (~421 GB/s aggregate measured); 32KB lines
# trip a slow path on one engine. Interleaved A/B benching showed
# uniform 4096 beats both 8192-based supertiles and tapered tails.
TAPER = [4096] * 8
DMA_C = 4096
DATA_BUFS = 4
F32 = mybir.dt.float32
assert sum(TAPER) == N


def _build(
    taper=None,
    data_bufs: int = DATA_BUFS,
    dma_c: int = DMA_C,
    dma_reverse: bool = False,
) -> bass.Bass:
    taper = list(TAPER if taper is None else taper)
    n_tiles = len(taper)
    offs = [sum(taper[:i]) for i in range(n_tiles)]
    nc = bacc.Bacc(
        "TRN2", target_bir_lowering=False, debug=False, num_devices=N_CORES
    )
    x = nc.dram_tensor("x", [ROWS_PER_CORE, N], F32, kind="ExternalInput").ap()
    out = nc.dram_tensor("out", [ROWS_PER_CORE, K], F32, kind="ExternalOutput").ap()

    with ExitStack() as ctx:
        tc = ctx.enter_context(tile.TileContext(nc))
        data_pool = ctx.enter_context(tc.tile_pool(name="data", bufs=data_bufs))
        small_pool = ctx.enter_context(tc.tile_pool(name="small", bufs=2 * N_BLOCKS))

        for b in range(N_BLOCKS):
            rows = slice(b * P, (b + 1) * P)
            cands = small_pool.tile([P, K * n_tiles], F32, tag="cands")
            for t, (off, sz) in enumerate(zip(offs, taper)):
                d = data_pool.tile([P, sz], F32, tag="data")
                for c0 in range(0, sz, dma_c):
                    c1 = min(c0 + dma_c, sz)
                    nc.sync.dma_start(
                        d[:, c0:c1], x[rows, off + c0 : off + c1]
                    )
                nc.vector.max(cands[:, t * K : (t + 1) * K], d[:])
            top = small_pool.tile([P, K], F32, tag="top")
            nc.vector.max(top[:], cands[:])
            if dma_reverse:
                # Reverse to ascending on the DMA's SBUF-read side (8
                # elements/partition, descriptor cost is negligible).
                nc.sync.dma_start(out[rows, :], top[:, ::-1])
            else:
                asc = small_pool.tile([P, K], F32, tag="asc")
                nc.vector.tensor_copy(asc[:], top[:, ::-1])
                nc.sync.dma_start(out[rows, :], asc[:])

    nc.compile()
    return nc


def _build_raw(taper=None, data_bufs: int = DATA_BUFS, dma_c: int = DMA_C) -> bass.Bass:
    """Manual-semaphore variant: no TileContext, so none of its
    EVSEM-butterfly preamble/exit barriers. Sync issues loads, Vector
    does the max8 chain, Scalar issues stores and holds the final
    completion wait."""
    taper = list(TAPER if taper is None else taper)
    n_tiles = len(taper)
    offs = [sum(taper[:i]) for i in range(n_tiles)]
    super_c = max(taper)
    nc = bacc.Bacc(
        "TRN2", target_bir_lowering=False, debug=False, num_devices=N_CORES
    )
    x = nc.dram_tensor("x", [ROWS_PER_CORE, N], F32, kind="ExternalInput").ap()
    out = nc.dram_tensor("out", [ROWS_PER_CORE, K], F32, kind="ExternalOutput").ap()

    # (block, tile_idx, col_off, cols, n_chunks) in stream order
    tiles_flat = []
    for b in range(N_BLOCKS):
        for t, (off, sz) in enumerate(zip(offs, taper)):
            nch = (sz + dma_c - 1) // dma_c
            tiles_flat.append((b, t, off, sz, nch))
    # Per-buffer-slot load semaphores: a single counting sem across all
    # tiles would be racy (concurrent chunk DMAs from different tiles
    # can mix to hit a threshold), but per-slot counts only saturate
    # when every chunk of that slot's latest tile has landed, because
    # the next tile on the slot isn't issued until the current one is
    # consumed (vd gate).
    slot_thresh = [0] * data_bufs
    tile_thresh = []
    for i, tf in enumerate(tiles_flat):
        s = i % data_bufs
        slot_thresh[s] += 16 * tf[4]
        tile_thresh.append(slot_thresh[s])

    with ExitStack() as ctx:
        block = ctx.enter_context(nc.Block())
        ld = [
            ctx.enter_context(nc.semaphore(f"ld{s}")) for s in range(data_bufs)
        ]
        vd = ctx.enter_context(nc.semaphore("vd"))
        fin = ctx.enter_context(nc.semaphore("fin"))
        res = ctx.enter_context(nc.semaphore("res"))
        st = ctx.enter_context(nc.semaphore("st"))
        data = [
            ctx.enter_context(nc.sbuf_tensor(f"data{i}", [P, super_c], F32))
            for i in range(data_bufs)
        ]
        cands = [
            ctx.enter_context(nc.sbuf_tensor(f"cands{b}", [P, K * n_tiles], F32))
            for b in range(N_BLOCKS)
        ]
        top = [
            ctx.enter_context(nc.sbuf_tensor(f"top{b}", [P, K], F32))
            for b in range(N_BLOCKS)
        ]
        asc = [
            ctx.enter_context(nc.sbuf_tensor(f"asc{b}", [P, K], F32))
            for b in range(N_BLOCKS)
        ]

        @block.sync
        def _(sync: bass.BassEngine):
            for i, (b, t, off, sz, nch) in enumerate(tiles_flat):
                buf = data[i % data_bufs]
                rows = slice(b * P, (b + 1) * P)
                if i >= data_bufs:
                    sync.wait_ge(vd, i - data_bufs + 1)
                for c0 in range(0, sz, dma_c):
                    c1 = min(c0 + dma_c, sz)
                    sync.dma_start(
                        out=buf[:, c0:c1], in_=x[rows, off + c0 : off + c1]
                    ).then_inc(ld[i % data_bufs], 16)

        @block.vector
        def _(vec: bass.BassVectorEngine):
            for i, (b, t, off, sz, nch) in enumerate(tiles_flat):
                buf = data[i % data_bufs]
                vec.wait_ge(ld[i % data_bufs], tile_thresh[i])
                vec.max(cands[b][:, t * K : (t + 1) * K], buf[:, :sz]).then_inc(
                    vd, 1
                )
                if t == n_tiles - 1:
                    # DVE writes drain asynchronously: same-engine RAW
                    # needs a sem wait for visibility, not just program
                    # order.
                    vec.wait_ge(vd, n_tiles * (b + 1))
                    vec.max(top[b][:], cands[b][:]).then_inc(fin, 1)
                    vec.wait_ge(fin, b + 1)
                    vec.tensor_copy(asc[b][:], top[b][:, ::-1]).then_inc(res, 1)

        @block.scalar
        def _(sc: bass.BassEngine):
            for b in range(N_BLOCKS):
                rows = slice(b * P, (b + 1) * P)
                sc.wait_ge(res, b + 1)
                sc.dma_start(out=out[rows, :], in_=asc[b][:]).then_inc(st, 16)
            sc.wait_ge(st, 16 * N_BLOCKS)

    nc.compile()
    return nc


def kernel(x: np.ndarray, k) -> np.ndarray:
    k = int(np.asarray(k))
    assert k == K, f"kernel hardcoded for k={K}, got {k}"
    assert x.shape == (B, N), x.shape
    x = np.ascontiguousarray(x, dtype=np.float32)

    nc = _build()
    in_maps = [
        {"x": x[c * ROWS_PER_CORE : (c + 1) * ROWS_PER_CORE]} for c in range(N_CORES)
    ]
    res = run_bass_kernel_spmd(nc, in_maps, list(range(N_CORES)))
    out = np.concatenate([res.results[c]["out"] for c in range(N_CORES)], axis=0)
    return np.asarray(out, dtype=np.float32)


if __name__ == "__main__":
    rng = np.random.default_rng(0)
    xs = rng.standard_normal((B, N), dtype=np.float32)
    got = kernel(xs, 8)
    want = np.sort(xs, axis=1)[:, -K:]
    err = np.max(np.abs(got - want))
    print("absmax err:", err)
